# revision 2
# baseline (speedup 1.0000x reference)
"""LMU (Legendre Memory Unit) Trainium2 Bass kernel.

Full-input contract: kernel(**inputs) takes the unsharded inputs from
setup_inputs() and returns the full (64, 2048, 512) output.

Algorithm: the per-step LMU update

    u = x_t@ie + h@he + m@me
    m' = m + m@AT + u@BT          (= m@Ad^T + u@BT)
    h' = tanh(x_t@ik + h@hk + m'@mk)

collapses (by substituting u and m') into ONE affine recurrence

    z' = [tanh | id]( pc_t + z @ W ),   z = [h(512); m(256)]

with a fixed (768,768) matrix W and a per-step input projection
pc_t = x_t @ Wx (256,768), both computed host-side from the weights.
The x-projection pc is computed on-device by a dense GEMM (chunked),
and the scan runs 2048 sequential 768x768 GEMM steps with N=8 (the
per-core batch).  Sharding: data-parallel batch 64 -> 8 cores x 8.

Scan matmuls are LDWEIGHTS-bound (the whole W streams through the PE
array every step while the moving operand is only 8 batch columns), so
the W tiles are split into col_split column strips executed on the
column-tiled PE (128x32 mode): the 4 strips' weight loads proceed
concurrently on separate XBUSes, cutting the per-step weight-load wall
by up to 4x.  K-tiles are issued m-part-first so the next step's
matmuls can start while tanh finishes, and PSUM accumulators ping-pong
between even/odd steps to avoid write-after-read stalls.

All matmuls keep the state transposed (zT: [768 part, 8 batch]) so the
tanh/adds stay on 128-partition tiles and no transposes are needed.
Scan is f32 throughout: bf16 anywhere in the recurrent path loses
~10% max-rel (the Legendre dynamics are marginally stable).  The
one-shot pc GEMM runs in float32r (full-rate fp32 mode).
"""

import os
import numpy as np
from contextlib import ExitStack

import concourse.bass as bass
import concourse.bacc as bacc
import concourse.tile as tile
import concourse.mybir as mybir
from concourse.bass_utils import run_bass_kernel_spmd

F32 = mybir.dt.float32
F32R = mybir.dt.float32r

B = 8          # batch per core
NCORES = 8
D = 256        # input dim
H = 512        # hidden units
MO = 256       # memory order
Z = H + MO     # 768 stacked state
KT = Z // 128  # 6 K-tiles
MT = Z // 128  # 6 M-tiles
HT = H // 128  # 4 h tiles

_cache = {}


def _build(T: int, cs: int = 4, ct: int = 32, korder: bool = True,
           pp: bool = True, pcdt: str = "f32r", repeat: int = 1):
    """Build the per-core Bass program for sequence length T."""
    assert T % (2 * ct) == 0
    cw = 128 // cs
    pdt = F32R if pcdt == "f32r" else F32
    nc = bacc.Bacc("TRN2", target_bir_lowering=False, debug=False)

    x_d = nc.dram_tensor("x", [B, T, D], pdt, kind="ExternalInput")
    w_d = nc.dram_tensor("W", [Z, Z], F32, kind="ExternalInput")
    wx_d = nc.dram_tensor("Wx", [D, Z], pdt, kind="ExternalInput")
    out_d = nc.dram_tensor("out", [B, T, H], F32, kind="ExternalOutput")

    with tile.TileContext(nc) as tc, ExitStack() as ctx:
        const = ctx.enter_context(tc.tile_pool(name="const", bufs=1))
        w_sb = const.tile([128, KT * Z], F32)     # tile (kt,mt) at (kt*MT+mt)*128
        wx_sb = const.tile([128, 2 * Z], pdt)
        xbuf = [const.tile([128, ct * 16], pdt, name=f"x{i}", tag=f"x{i}") for i in range(2)]
        pcbuf = [const.tile([128, MT * ct * B], F32, name=f"pc{i}", tag=f"pc{i}") for i in range(2)]
        hbuf = [const.tile([128, ct * 4 * B], F32, name=f"h{i}", tag=f"h{i}") for i in range(2)]
        mbuf = [const.tile([128, 2 * B], F32, name=f"m{i}", tag=f"m{i}") for i in range(2)]

        pspool = ctx.enter_context(tc.tile_pool(name="ps", bufs=1, space="PSUM"))
        ps_scan = pspool.tile([128, MT * 512], F32)   # 6 banks, one per M-tile
        pcps = ctx.enter_context(tc.tile_pool(name="pcps", bufs=2, space="PSUM"))
        tmp_pool = ctx.enter_context(tc.tile_pool(name="tmp", bufs=4))

        # --- prologue: weights + state init ---
        for kt in range(KT):
            nc.sync.dma_start(
                w_sb[:, kt * Z:(kt + 1) * Z], w_d.ap()[kt * 128:(kt + 1) * 128, :])
        for k2 in range(2):
            nc.sync.dma_start(
                wx_sb[:, k2 * Z:(k2 + 1) * Z], wx_d.ap()[k2 * 128:(k2 + 1) * 128, :])
        nc.vector.memset(mbuf[1][:], 0.0)                       # m_{-1} = 0
        nc.vector.memset(hbuf[1][:, (ct - 1) * 32:ct * 32], 0.0)  # h_{-1} = 0

        def dma_x(xb, toff):
            dstv = xb[:].rearrange("p (t w) -> p t w", t=ct, w=16)
            for k2 in range(2):
                for b in range(B):
                    src = x_d.ap()[b, bass.ds(toff, ct),
                                   k2 * 128:(k2 + 1) * 128].rearrange("t p -> p t")
                    nc.sync.dma_start(dstv[:, :, k2 * B + b], src)

        def pc_gemm(xb, pcb):
            xv = xb[:].rearrange("p (t k b) -> p t k b", t=ct, k=2, b=B)
            ncol = ct * B
            nsub = max(1, ncol // 512)
            sub = ncol // nsub
            for mt in range(MT):
                for ns in range(nsub):
                    ps = pcps.tile([128, sub], F32, name="pcp", tag="pcps")
                    for k2 in range(2):
                        nc.tensor.matmul(
                            ps[:],
                            wx_sb[:, k2 * Z + mt * 128: k2 * Z + (mt + 1) * 128],
                            xv[:, ns * (sub // B):(ns + 1) * (sub // B), k2, :],
                            start=(k2 == 0), stop=(k2 == 1))
                    nc.scalar.copy(
                        pcb[:, mt * ct * B + ns * sub: mt * ct * B + (ns + 1) * sub],
                        ps[:])

        kt_seq = ([HT, HT + 1] + list(range(HT))) if korder else list(range(KT))
        mt_seq = ([HT, HT + 1] + list(range(HT))) if korder else list(range(MT))

        def scan_chunk(hb, hb_prev, pcb):
            psv = ps_scan[:].rearrange("p (m x) -> p m x", m=MT, x=512)
            pcv = pcb[:].rearrange("p (m t b) -> p m t b", m=MT, t=ct, b=B)
            for t in range(ct):
                psoff = (t % 2) * 256 if pp else 0
                hprev = (hb_prev if t == 0 else hb)[
                    :, ((t - 1) % ct) * 32:(((t - 1) % ct) + 1) * 32]
                m_in = mbuf[1 - (t % 2)]
                m_out = mbuf[t % 2]
                for kt in kt_seq:
                    if kt < HT:
                        rhs = hprev[:, kt * B:(kt + 1) * B]
                    else:
                        rhs = m_in[:, (kt - HT) * B:(kt - HT + 1) * B]
                    for mt in mt_seq:
                        w_tile = w_sb[:, (kt * MT + mt) * 128:(kt * MT + mt + 1) * 128]
                        for cj in range(cs):
                            nc.tensor.matmul(
                                ps_scan[cj * cw:(cj + 1) * cw,
                                        mt * 512 + psoff: mt * 512 + psoff + B],
                                w_tile[:, cj * cw:(cj + 1) * cw], rhs,
                                start=(kt == kt_seq[0]), stop=(kt == kt_seq[-1]),
                                tile_position=(0, cj * cw) if cs > 1 else None)
                # m' = psum_m + pc_m   (issued first: next step's kt=4,5 need it)
                nc.vector.tensor_add(
                    m_out[:].rearrange("p (m b) -> p m b", m=2, b=B),
                    psv[:, HT:MT, psoff:psoff + B], pcv[:, HT:MT, t, :])
                # h' = tanh(psum_h + pc_h)
                tmp = tmp_pool.tile([128, 4 * B], F32, name="tmph", tag="tmph")
                nc.vector.tensor_add(
                    tmp[:].rearrange("p (m b) -> p m b", m=HT, b=B),
                    psv[:, 0:HT, psoff:psoff + B], pcv[:, 0:HT, t, :])
                nc.scalar.activation(
                    hb[:, t * 32:(t + 1) * 32], tmp[:],
                    mybir.ActivationFunctionType.Tanh)

        def dma_out(hb, toff):
            hv = hb[:].rearrange("p (t w) -> p t w", t=ct, w=4 * B)
            for mt in range(HT):
                for b in range(B):
                    dst = out_d.ap()[b, bass.ds(toff, ct),
                                     mt * 128:(mt + 1) * 128].rearrange("t p -> p t")
                    nc.sync.dma_start(dst, hv[:, :, mt * B + b])

        def body(toff):
            dma_x(xbuf[0], toff)
            pc_gemm(xbuf[0], pcbuf[0])
            dma_x(xbuf[1], toff + ct)
            scan_chunk(hbuf[0], hbuf[1], pcbuf[0])
            dma_out(hbuf[0], toff)
            pc_gemm(xbuf[1], pcbuf[1])
            scan_chunk(hbuf[1], hbuf[0], pcbuf[1])
            dma_out(hbuf[1], toff + ct)

        if repeat > 1:
            with tc.For_i(0, repeat) as _r:
                with tc.For_i(0, T, 2 * ct) as toff:
                    body(toff)
        else:
            with tc.For_i(0, T, 2 * ct) as toff:
                body(toff)

    nc.compile()
    return nc


def _host_weights(inputs):
    """Fold all the LMU weights into W (768,768) and Wx (256,768), f64 host math."""
    ie = np.asarray(inputs["input_encoders"], np.float64)    # (256,1)
    he = np.asarray(inputs["hidden_encoders"], np.float64)   # (512,1)
    me = np.asarray(inputs["memory_encoders"], np.float64)   # (256,1)
    ik = np.asarray(inputs["input_kernel"], np.float64)      # (256,512)
    hk = np.asarray(inputs["hidden_kernel"], np.float64)     # (512,512)
    mk = np.asarray(inputs["memory_kernel"], np.float64)     # (256,512)
    AT = np.asarray(inputs["AT"], np.float64)                # (256,256)
    BT = np.asarray(inputs["BT"], np.float64)                # (1,256)
    ATI = AT + np.eye(MO)            # = Ad^T
    mk2 = ATI @ mk                   # (256,512)
    g = BT @ mk                      # (1,512)
    W = np.zeros((Z, Z))
    W[0:H, 0:H] = hk + he @ g
    W[H:Z, 0:H] = mk2 + me @ g
    W[0:H, H:Z] = he @ BT
    W[H:Z, H:Z] = ATI + me @ BT
    Wx = np.zeros((D, Z))
    Wx[:, 0:H] = ik + ie @ g
    Wx[:, H:Z] = ie @ BT
    return W.astype(np.float32), Wx.astype(np.float32)


def kernel(**inputs):
    x = np.ascontiguousarray(np.asarray(inputs["x"], np.float32))
    Bfull, T, _ = x.shape
    W, Wx = _host_weights(inputs)

    cs = int(os.environ.get("LMU_COLSPLIT", "4"))
    ct = int(os.environ.get("LMU_CT", "32"))
    korder = os.environ.get("LMU_KORDER", "1") == "1"
    pp = os.environ.get("LMU_PP", "1") == "1"
    pcdt = os.environ.get("LMU_PCDT", "f32r")
    rep = int(os.environ.get("LMU_REPEAT", "1"))
    key = (T, cs, ct, korder, pp, pcdt, rep)
    if key not in _cache:
        _cache[key] = _build(T, cs=cs, ct=ct, korder=korder, pp=pp,
                             pcdt=pcdt, repeat=rep)
    nc = _cache[key]

    per = Bfull // NCORES
    in_maps = [
        {"x": np.ascontiguousarray(x[c * per:(c + 1) * per]), "W": W, "Wx": Wx}
        for c in range(NCORES)
    ]
    res = run_bass_kernel_spmd(nc, in_maps, core_ids=list(range(NCORES)))
    out = np.concatenate([r["out"] for r in res.results], axis=0)
    return out
